# revision 13
# baseline (speedup 1.0000x reference)
"""Trainium2 Bass kernel: batched multi-head attention with post-softmax
multiplicative key-mask (B=4, H=12, S=2048, Dk=Dv=64, fp32 I/O).

out = softmax(Q K^T / 8) * mask(k) @ V  ==  softmax(Q K^T / 8) @ (diag(mask) V)

Sharding: B*H = 48 heads, 6 per NeuronCore across 8 cores (no cross-core
communication).

Per-head on-chip algorithm (S^T layout so no giant transposes are needed):
  - Q, K are cast to bf16 and transposed to [d, q]/[d, k] "parity" layout
    via the DMA xbar transpose ([1024, 128] DRAM view -> [128, 1024] SBUF
    where rows 0:64 = even rows' transpose, 64:128 = odd rows').
  - S^T[k-chunk, q] = (K^T chunk).T @ Q^T tiles on TensorE (bf16, fp32 acc).
  - P^T = exp(S^T / 8) on ScalarE (exact fp32 spline), output bf16.
  - row sums (softmax denominators) = ones.T @ P^T accumulated over chunks
    (column-packed 4-wide on the PE array).
  - out^T[v, q] = sum_c V'[c].T-style accumulation: matmul(lhsT=V'chunk,
    rhs=P^T chunk) accumulated over 16 chunks (column-packed 2-wide),
    where V' = mask(k) * V in bf16.
  - normalize by 1/sums (reciprocal_approx_accurate + gpsimd
    partition_broadcast + one DVE multiply), PE-transpose back to [q, v],
    DMA out.
"""

import sys

for _p in ("/opt/trn_rl_repo", "/root/.axon_site/_ro/trn_rl_repo"):
    if _p not in sys.path:
        sys.path.insert(0, _p)

import numpy as np

import concourse.bass as bass
import concourse.mybir as mybir
import concourse.tile as tile
from concourse import bacc
from concourse.bass_utils import run_bass_kernel_spmd

FP32 = mybir.dt.float32
BF16 = mybir.dt.bfloat16
AF = mybir.ActivationFunctionType

S = 2048
D = 64
NCH = 16          # k chunks of 128
N_CORES = 8
HPC = 6           # heads per core (48 total)


def _build_head(tc, pools, O, Q, K, V, M, h, dbg=None):
    nc = tc.nc
    (ones_bf, ident, dramp, headp, ptp, pss, psav, pssum, pstr, outp, smallp) = pools

    # ---------------- phase 0: load + prep ----------------
    # bf16 staging in DRAM, natural [S, D] layout (gpsimd cast-DMA)
    qbf_d = dramp.tile([S, D], BF16, tag="qbf_d")
    kbf_d = dramp.tile([S, D], BF16, tag="kbf_d")
    nc.gpsimd.dma_start(out=qbf_d[:], in_=Q[h])
    nc.gpsimd.dma_start(out=kbf_d[:], in_=K[h])

    # xbar transpose -> parity layout:
    #   QT[c, r] (c<64):  Q^T[d=c, q=2r]   (even q)
    #   QT[c, r] (c>=64): Q^T[d=c-64, q=2r+1] (odd q)
    QT = headp.tile([128, S // 2], BF16, tag="QT")
    KT = headp.tile([128, S // 2], BF16, tag="KT")
    nc.sync.dma_start_transpose(QT[:], qbf_d[:].rearrange("(r two) d -> r (two d)", two=2))
    nc.sync.dma_start_transpose(KT[:], kbf_d[:].rearrange("(r two) d -> r (two d)", two=2))

    # swapped-half copy of QT (odd-q rows on top), for the cross-parity MMs
    QTs = headp.tile([128, S // 2], BF16, tag="QTs")
    nc.sync.dma_start(out=QTs[0:64, :], in_=QT[64:128, :])
    nc.sync.dma_start(out=QTs[64:128, :], in_=QT[0:64, :])

    if dbg is not None:
        nc.sync.dma_start(out=dbg["QT"][h], in_=QT[:])
        nc.sync.dma_start(out=dbg["QTs"][h], in_=QTs[:])
        nc.sync.dma_start(out=dbg["KT"][h], in_=KT[:])

    # V and mask, k-reordered to parity chunks: chunk c<8 holds k=256c+2i,
    # chunk c>=8 holds k=256(c-8)+2i+1  (i = partition row)
    vf32 = headp.tile([128, NCH * D], FP32, tag="vf32")
    mk = headp.tile([128, NCH], FP32, tag="mk")
    for par in range(2):
        nc.sync.dma_start(
            out=vf32[:, par * 8 * D:(par + 1) * 8 * D].rearrange(
                "p (c8 d) -> p c8 d", d=D),
            in_=V[h].rearrange("(c8 i two) d -> two i c8 d", two=2, i=128)[par],
        )
        nc.gpsimd.dma_start(
            out=mk[:, par * 8:(par + 1) * 8],
            in_=M[h].rearrange("(c8 i two) -> two i c8", two=2, i=128)[par],
        )
    # masked V in bf16 (mask fold exact: mask is 0.0/1.0)
    vp = headp.tile([128, NCH * D], BF16, tag="vp")
    for c in range(NCH):
        nc.vector.tensor_scalar_mul(
            vp[:, c * D:(c + 1) * D], vf32[:, c * D:(c + 1) * D], mk[:, c:c + 1]
        )

    if dbg is not None:
        nc.sync.dma_start(out=dbg["vp"][h], in_=vp[:])
        nc.sync.dma_start(out=dbg["mk"][h], in_=mk[:])
        nc.sync.dma_start(out=dbg["vf32"][h], in_=vf32[:])

    # ---------------- phase 1: S^T = K Q^T, P^T = exp(S^T/8) ----------------
    # P^T stored bf16: cols [c*2048 + 0:1024] even-q (q=2r), [+1024:2048] odd-q
    pt = ptp.tile([128, NCH * S], BF16, tag="pt")
    for c in range(NCH):
        base = 0 if c < 8 else 64
        lhsT = KT[base:base + 64, (c % 8) * 128:(c % 8 + 1) * 128]
        for half in range(2):
            if (half == 0) == (c < 8):
                rhs_t = QT  # natural parity
            else:
                rhs_t = QTs
            st = pss.tile([128, 1024], FP32, tag="st")
            for t in range(2):
                nc.tensor.matmul(
                    st[:, t * 512:(t + 1) * 512],
                    lhsT,
                    rhs_t[base:base + 64, t * 512:(t + 1) * 512],
                    start=True,
                    stop=True,
                    tile_position=(base, 0),
                )
            nc.scalar.activation(
                out=pt[:, c * S + half * 1024: c * S + half * 1024 + 1024],
                in_=st[:],
                func=AF.Exp,
                scale=0.125,
            )

    if dbg is not None:
        nc.sync.dma_start(out=dbg["pt0"][h], in_=pt[:, 0:S])
        nc.sync.dma_start(out=dbg["pt15"][h], in_=pt[:, 15 * S:16 * S])

    # ---------------- phase 2: sums, AV, normalize, transpose, store -------
    sums = pssum.tile([128, 512], FP32, tag="sums")       # rows 0/32/64/96
    for c in range(NCH):
        pcol = c * S
        for qt in range(4):
            nc.tensor.matmul(
                sums[qt * 32:qt * 32 + 1, :],
                ones_bf[:, 0:1],
                pt[:, pcol + qt * 512: pcol + (qt + 1) * 512],
                start=(c == 0),
                stop=(c == NCH - 1),
                tile_position=(0, qt * 32),
                skip_group_check=True,
            )
    avs = []
    for qt in range(4):
        av = psav.tile([64, 512], FP32, tag="av")
        avs.append(av)
        for c in range(NCH):
            nc.tensor.matmul(
                av[:],
                vp[:, c * D:(c + 1) * D],
                pt[:, c * S + qt * 512: c * S + (qt + 1) * 512],
                start=(c == 0),
                stop=(c == NCH - 1),
            )

    # softmax denominators: copy the 4 sum rows to SBUF, bounce through
    # DRAM into a [128, 16] per-partition layout (col = qt*4 + j, partition
    # = within-j-block index i, matching the transposed output tiles), then
    # one standard reciprocal op.
    rsb = smallp.tile([128, 512], FP32, tag="rsb")
    for qt in range(4):
        r = qt * 32
        nc.vector.tensor_copy(rsb[r:r + 1, :], sums[r:r + 1, :])
    sums_d = dramp.tile([4, 512], FP32, tag="sums_d")
    for qt in range(4):
        nc.sync.dma_start(out=sums_d[qt], in_=rsb[qt * 32:qt * 32 + 1, :])
    rs128 = smallp.tile([128, NCH], FP32, tag="rs128")
    nc.gpsimd.dma_start(
        out=rs128[:].rearrange("i (qt j) -> i qt j", qt=4),
        in_=sums_d[:].rearrange("qt (j i) -> i qt j", j=4),
    )
    rrec = smallp.tile([128, NCH], FP32, tag="rrec")
    nc.vector.reciprocal(rrec[:], rs128[:])

    if dbg is not None:
        nc.sync.dma_start(out=dbg["rsb"][h], in_=rsb[:])
        nc.sync.dma_start(out=dbg["rrec"][h], in_=rrec[:])

    # copy out^T psum to SBUF, PE-transpose back to [q, v], normalize by
    # 1/sums as a per-partition scalar during the PSUM->SBUF move
    ofin = outp.tile([128, NCH * D], FP32, tag="ofin")
    for qt in range(4):
        osb = outp.tile([64, 512], FP32, tag="osb")
        nc.vector.tensor_copy(osb[:], avs[qt][:])
        if dbg is not None and qt % 2 == 0:
            nc.sync.dma_start(out=dbg["osb"][h, qt // 2], in_=osb[:])
        # PE transpose [64, 128] -> [128, 64] per j-slice, then scale+copy
        for j in range(4):
            tr = pstr.tile([128, 64], FP32, tag="tr")
            nc.tensor.transpose(
                tr[:],
                osb[:, j * 128:(j + 1) * 128],
                ident[0:64, :],
            )
            col = qt * 4 + j
            nc.vector.tensor_scalar_mul(
                ofin[:, col * D:(col + 1) * D], tr[:], rrec[:, col:col + 1]
            )

    # store: ofin col (qt*4+j)*64+v, partition i  ->  O[h, q, v],
    # q = 2*((qt%2)*512 + j*128 + i) + (qt//2)
    for half in range(2):
        nc.sync.dma_start(
            out=O[h].rearrange("(qt j i two) v -> two i qt j v", qt=2, j=4, two=2)[half],
            in_=ofin[:].rearrange("i (half qt j v) -> half i qt j v", half=2, qt=2, j=4)[half],
        )


def build_attention(hpc=HPC, n_cores=N_CORES, trn_type="TRN2", debug_outputs=False):
    nc = bacc.Bacc(
        trn_type,
        target_bir_lowering=False,
        debug=False,
        num_devices=n_cores,
    )
    Q = nc.dram_tensor("Q", [hpc, S, D], FP32, kind="ExternalInput").ap()
    K = nc.dram_tensor("K", [hpc, S, D], FP32, kind="ExternalInput").ap()
    V = nc.dram_tensor("V", [hpc, S, D], FP32, kind="ExternalInput").ap()
    M = nc.dram_tensor("MASK", [hpc, S], FP32, kind="ExternalInput").ap()
    O = nc.dram_tensor("O", [hpc, S, D], FP32, kind="ExternalOutput").ap()
    dbg = None
    if debug_outputs:
        dbg = {
            "QT": nc.dram_tensor("DBG_QT", [hpc, 128, S // 2], BF16, kind="ExternalOutput").ap(),
            "QTs": nc.dram_tensor("DBG_QTs", [hpc, 128, S // 2], BF16, kind="ExternalOutput").ap(),
            "KT": nc.dram_tensor("DBG_KT", [hpc, 128, S // 2], BF16, kind="ExternalOutput").ap(),
            "pt0": nc.dram_tensor("DBG_pt0", [hpc, 128, S], BF16, kind="ExternalOutput").ap(),
            "pt15": nc.dram_tensor("DBG_pt15", [hpc, 128, S], BF16, kind="ExternalOutput").ap(),
            "rsb": nc.dram_tensor("DBG_rsb", [hpc, 128, 512], FP32, kind="ExternalOutput").ap(),
            "vp": nc.dram_tensor("DBG_vp", [hpc, 128, NCH * D], BF16, kind="ExternalOutput").ap(),
            "mk": nc.dram_tensor("DBG_mk", [hpc, 128, NCH], FP32, kind="ExternalOutput").ap(),
            "vf32": nc.dram_tensor("DBG_vf32", [hpc, 128, NCH * D], FP32, kind="ExternalOutput").ap(),
            "rrec": nc.dram_tensor("DBG_rrec", [hpc, 128, NCH], FP32, kind="ExternalOutput").ap(),
            "osb": nc.dram_tensor("DBG_osb", [hpc, 2, 64, 512], FP32, kind="ExternalOutput").ap(),
        }

    with tile.TileContext(nc) as tc:
        with (
            tc.tile_pool(name="const", bufs=1) as constp,
            tc.tile_pool(name="dram", bufs=2, space="DRAM") as dramp,
            tc.tile_pool(name="heads", bufs=2) as headp,
            tc.tile_pool(name="ptp", bufs=2) as ptp,
            tc.tile_pool(name="pss", bufs=2, space="PSUM") as pss,
            tc.tile_pool(name="psav", bufs=2, space="PSUM") as psav,
            tc.tile_pool(name="pssum", bufs=1, space="PSUM") as pssum,
            tc.tile_pool(name="pstr", bufs=1, space="PSUM") as pstr,
            tc.tile_pool(name="outp", bufs=2) as outp,
            tc.tile_pool(name="smallp", bufs=2) as smallp,
        ):
            ones_bf = constp.tile([128, 1], BF16, tag="ones_bf")
            nc.vector.memset(ones_bf[:], 1.0)
            ident = constp.tile([128, D], FP32, tag="ident")
            nc.gpsimd.memset(ident[:], 0.0)
            from concourse.masks import make_identity
            make_identity(nc, ident[0:64, :], nomemset=True)
            make_identity(nc, ident[64:128, :], nomemset=True)

            pools = (ones_bf, ident, dramp, headp, ptp, pss, psav, pssum, pstr, outp, smallp)
            for h in range(hpc):
                _build_head(tc, pools, O, Q, K, V, M, h, dbg=dbg)

    nc.compile()
    return nc


_CACHE = {}


def _get_nc(hpc=HPC):
    if hpc not in _CACHE:
        _CACHE[hpc] = build_attention(hpc=hpc)
    return _CACHE[hpc]


def _shard(Q, K, V, mask_out):
    B, H, S_, Dk = Q.shape
    BH = B * H
    hpc = BH // N_CORES
    q = np.ascontiguousarray(np.asarray(Q, dtype=np.float32).reshape(BH, S_, Dk))
    k = np.ascontiguousarray(np.asarray(K, dtype=np.float32).reshape(BH, S_, Dk))
    v = np.ascontiguousarray(np.asarray(V, dtype=np.float32).reshape(BH, S_, Dk))
    m = np.ascontiguousarray(np.asarray(mask_out, dtype=np.float32).reshape(BH, S_))
    in_maps = []
    for i in range(N_CORES):
        sl = slice(i * hpc, (i + 1) * hpc)
        in_maps.append({"Q": q[sl], "K": k[sl], "V": v[sl], "MASK": m[sl]})
    return in_maps, hpc


def kernel(Q, K, V, mask_out):
    B, H, S_, Dk = np.asarray(Q).shape
    in_maps, hpc = _shard(Q, K, V, mask_out)
    nc = _get_nc(hpc)
    res = run_bass_kernel_spmd(nc, in_maps, list(range(N_CORES)))
    O = np.concatenate([res.results[i]["O"] for i in range(N_CORES)], axis=0)
    return np.ascontiguousarray(O.reshape(B, H, S_, Dk).astype(np.float32))


def bench(Q, K, V, mask_out, trace=True):
    """Run once with tracing; returns (output, exec_time_ns, profile)."""
    in_maps, hpc = _shard(Q, K, V, mask_out)
    nc = _get_nc(hpc)
    res = run_bass_kernel_spmd(nc, in_maps, list(range(N_CORES)), trace=trace)
    B, H = 4, 12
    O = np.concatenate([res.results[i]["O"] for i in range(N_CORES)], axis=0)
    out = np.ascontiguousarray(O.reshape(B, H, S, D).astype(np.float32))
    return out, res.exec_time_ns, res


# revision 14
# speedup vs baseline: 1.0714x; 1.0714x over previous
"""Trainium2 Bass kernel: batched multi-head attention with post-softmax
multiplicative key-mask (B=4, H=12, S=2048, Dk=Dv=64, fp32 I/O).

out = softmax(Q K^T / 8) * mask(k) @ V  ==  softmax(Q K^T / 8) @ (diag(mask) V)

Sharding: B*H = 48 heads, 6 per NeuronCore across 8 cores (no cross-core
communication).

Per-head on-chip algorithm (S^T layout so no giant transposes are needed):
  - Q, K are cast to bf16 and transposed to [d, q]/[d, k] "parity" layout
    via the DMA xbar transpose ([1024, 128] DRAM view -> [128, 1024] SBUF
    where rows 0:64 = even rows' transpose, 64:128 = odd rows').
  - S^T[k-chunk, q] = (K^T chunk).T @ Q^T tiles on TensorE (bf16, fp32 acc).
  - P^T = exp(S^T / 8) on ScalarE (exact fp32 spline), output bf16.
  - row sums (softmax denominators) = ones.T @ P^T accumulated over chunks
    (column-packed 4-wide on the PE array).
  - out^T[v, q] = sum_c V'[c].T-style accumulation: matmul(lhsT=V'chunk,
    rhs=P^T chunk) accumulated over 16 chunks (column-packed 2-wide),
    where V' = mask(k) * V in bf16.
  - normalize by 1/sums (reciprocal_approx_accurate + gpsimd
    partition_broadcast + one DVE multiply), PE-transpose back to [q, v],
    DMA out.
"""

import sys

for _p in ("/opt/trn_rl_repo", "/root/.axon_site/_ro/trn_rl_repo"):
    if _p not in sys.path:
        sys.path.insert(0, _p)

import numpy as np

import concourse.bass as bass
import concourse.mybir as mybir
import concourse.tile as tile
from concourse import bacc
from concourse.bass_utils import run_bass_kernel_spmd

FP32 = mybir.dt.float32
BF16 = mybir.dt.bfloat16
AF = mybir.ActivationFunctionType

S = 2048
D = 64
NCH = 16          # k chunks of 128
N_CORES = 8
HPC = 6           # heads per core (48 total)


def _build_head(tc, pools, O, Q, K, V, M, h, dbg=None):
    nc = tc.nc
    (ones_bf, ident, dramp, headp, ptp, pss, psav, pssum, pstr, outp, smallp) = pools

    # ---------------- phase 0: load + prep ----------------
    # bf16 staging in DRAM, natural [S, D] layout (gpsimd cast-DMA)
    qbf_d = dramp.tile([S, D], BF16, tag="qbf_d")
    kbf_d = dramp.tile([S, D], BF16, tag="kbf_d")
    nc.gpsimd.dma_start(out=qbf_d[:], in_=Q[h])
    nc.gpsimd.dma_start(out=kbf_d[:], in_=K[h])

    # xbar transpose -> parity layout:
    #   QT[c, r] (c<64):  Q^T[d=c, q=2r]   (even q)
    #   QT[c, r] (c>=64): Q^T[d=c-64, q=2r+1] (odd q)
    QT = headp.tile([128, S // 2], BF16, tag="QT")
    KT = headp.tile([128, S // 2], BF16, tag="KT")
    nc.sync.dma_start_transpose(QT[:], qbf_d[:].rearrange("(r two) d -> r (two d)", two=2))
    nc.sync.dma_start_transpose(KT[:], kbf_d[:].rearrange("(r two) d -> r (two d)", two=2))

    # swapped-half copy of QT (odd-q rows on top), for the cross-parity MMs
    QTs = headp.tile([128, S // 2], BF16, tag="QTs")
    nc.sync.dma_start(out=QTs[0:64, :], in_=QT[64:128, :])
    nc.sync.dma_start(out=QTs[64:128, :], in_=QT[0:64, :])

    if dbg is not None:
        nc.sync.dma_start(out=dbg["QT"][h], in_=QT[:])
        nc.sync.dma_start(out=dbg["QTs"][h], in_=QTs[:])
        nc.sync.dma_start(out=dbg["KT"][h], in_=KT[:])

    # V and mask, k-reordered to parity chunks: chunk c<8 holds k=256c+2i,
    # chunk c>=8 holds k=256(c-8)+2i+1  (i = partition row)
    vf32 = headp.tile([128, NCH * D], FP32, tag="vf32")
    mk = headp.tile([128, NCH], FP32, tag="mk")
    for par in range(2):
        nc.sync.dma_start(
            out=vf32[:, par * 8 * D:(par + 1) * 8 * D].rearrange(
                "p (c8 d) -> p c8 d", d=D),
            in_=V[h].rearrange("(c8 i two) d -> two i c8 d", two=2, i=128)[par],
        )
        nc.gpsimd.dma_start(
            out=mk[:, par * 8:(par + 1) * 8],
            in_=M[h].rearrange("(c8 i two) -> two i c8", two=2, i=128)[par],
        )
    # masked V in bf16 (mask fold exact: mask is 0.0/1.0)
    vp = headp.tile([128, NCH * D], BF16, tag="vp")
    for c in range(NCH):
        nc.vector.tensor_scalar_mul(
            vp[:, c * D:(c + 1) * D], vf32[:, c * D:(c + 1) * D], mk[:, c:c + 1]
        )

    if dbg is not None:
        nc.sync.dma_start(out=dbg["vp"][h], in_=vp[:])
        nc.sync.dma_start(out=dbg["mk"][h], in_=mk[:])
        nc.sync.dma_start(out=dbg["vf32"][h], in_=vf32[:])

    # ---------------- phase 1: S^T = K Q^T, P^T = exp(S^T/8) ----------------
    # P^T stored bf16: cols [c*2048 + 0:1024] even-q (q=2r), [+1024:2048] odd-q
    pt = ptp.tile([128, NCH * S], BF16, tag="pt")
    qk_order = []
    for c8 in range(8):
        for half in range(2):
            qk_order.append((c8, half))
            qk_order.append((c8 + 8, half))
    for c, half in qk_order:
        base = 0 if c < 8 else 64
        lhsT = KT[base:base + 64, (c % 8) * 128:(c % 8 + 1) * 128]
        if True:
            if (half == 0) == (c < 8):
                rhs_t = QT  # natural parity
            else:
                rhs_t = QTs
            st = pss.tile([128, 1024], FP32, tag="st")
            for t in range(2):
                nc.tensor.matmul(
                    st[:, t * 512:(t + 1) * 512],
                    lhsT,
                    rhs_t[base:base + 64, t * 512:(t + 1) * 512],
                    start=True,
                    stop=True,
                    tile_position=(base, 0),
                )
            nc.scalar.activation(
                out=pt[:, c * S + half * 1024: c * S + half * 1024 + 1024],
                in_=st[:],
                func=AF.Exp,
                scale=0.125,
            )

    if dbg is not None:
        nc.sync.dma_start(out=dbg["pt0"][h], in_=pt[:, 0:S])
        nc.sync.dma_start(out=dbg["pt15"][h], in_=pt[:, 15 * S:16 * S])

    # ---------------- phase 2: sums, AV, normalize, transpose, store -------
    sums = pssum.tile([128, 512], FP32, tag="sums")       # rows 0/32/64/96
    for c in range(NCH):
        pcol = c * S
        for qt in range(4):
            nc.tensor.matmul(
                sums[qt * 32:qt * 32 + 1, :],
                ones_bf[:, 0:1],
                pt[:, pcol + qt * 512: pcol + (qt + 1) * 512],
                start=(c == 0),
                stop=(c == NCH - 1),
                tile_position=(0, qt * 32),
                skip_group_check=True,
            )
    av0 = psav.tile([128, 512], FP32, tag="av")    # rows 0:64 = qt0, 64:128 = qt1
    av1 = psav.tile([128, 512], FP32, tag="av")    # rows 0:64 = qt2, 64:128 = qt3
    for c in range(NCH):
        for pair, av in ((0, av0), (1, av1)):
            for slot in range(2):
                qt = pair * 2 + slot
                nc.tensor.matmul(
                    av[slot * 64:(slot + 1) * 64, :],
                    vp[:, c * D:(c + 1) * D],
                    pt[:, c * S + qt * 512: c * S + (qt + 1) * 512],
                    start=(c == 0),
                    stop=(c == NCH - 1),
                    tile_position=(0, slot * 64),
                    skip_group_check=True,
                )

    # softmax denominators: copy the 4 sum rows to SBUF, bounce through
    # DRAM into a [128, 16] per-partition layout (col = qt*4 + j, partition
    # = within-j-block index i, matching the transposed output tiles), then
    # one standard reciprocal op.
    rsb = smallp.tile([128, 512], FP32, tag="rsb")
    for qt in range(4):
        r = qt * 32
        nc.vector.tensor_copy(rsb[r:r + 1, :], sums[r:r + 1, :])
    sums_d = dramp.tile([4, 512], FP32, tag="sums_d")
    for qt in range(4):
        nc.sync.dma_start(out=sums_d[qt], in_=rsb[qt * 32:qt * 32 + 1, :])
    rs128 = smallp.tile([128, NCH], FP32, tag="rs128")
    nc.gpsimd.dma_start(
        out=rs128[:].rearrange("i (qt j) -> i qt j", qt=4),
        in_=sums_d[:].rearrange("qt (j i) -> i qt j", j=4),
    )
    rrec = smallp.tile([128, NCH], FP32, tag="rrec")
    nc.vector.reciprocal(rrec[:], rs128[:])

    if dbg is not None:
        nc.sync.dma_start(out=dbg["rsb"][h], in_=rsb[:])
        nc.sync.dma_start(out=dbg["rrec"][h], in_=rrec[:])

    # copy out^T psum to SBUF, PE-transpose back to [q, v], normalize by
    # 1/sums as a per-partition scalar during the PSUM->SBUF move
    ofin = outp.tile([128, NCH * D], FP32, tag="ofin")
    for pair, av in ((0, av0), (1, av1)):
        osb = outp.tile([128, 512], FP32, tag="osb")
        nc.vector.tensor_copy(osb[:], av[:])
        if dbg is not None:
            nc.sync.dma_start(out=dbg["osb"][h, pair], in_=osb[0:64, :])
        # PE transpose [64, 128] -> [128, 64] per j-slice, then scale+copy
        for slot in range(2):
            qt = pair * 2 + slot
            rb = slot * 64
            for j in range(4):
                tr = pstr.tile([128, 64], FP32, tag="tr")
                nc.tensor.transpose(
                    tr[:],
                    osb[rb:rb + 64, j * 128:(j + 1) * 128],
                    ident[rb:rb + 64, :],
                    tile_position=(rb, 0),
                )
                col = qt * 4 + j
                nc.vector.tensor_scalar_mul(
                    ofin[:, col * D:(col + 1) * D], tr[:], rrec[:, col:col + 1]
                )

    # store: ofin col (qt*4+j)*64+v, partition i  ->  O[h, q, v],
    # q = 2*((qt%2)*512 + j*128 + i) + (qt//2)
    for half in range(2):
        nc.sync.dma_start(
            out=O[h].rearrange("(qt j i two) v -> two i qt j v", qt=2, j=4, two=2)[half],
            in_=ofin[:].rearrange("i (half qt j v) -> half i qt j v", half=2, qt=2, j=4)[half],
        )


def build_attention(hpc=HPC, n_cores=N_CORES, trn_type="TRN2", debug_outputs=False):
    nc = bacc.Bacc(
        trn_type,
        target_bir_lowering=False,
        debug=False,
        num_devices=n_cores,
    )
    Q = nc.dram_tensor("Q", [hpc, S, D], FP32, kind="ExternalInput").ap()
    K = nc.dram_tensor("K", [hpc, S, D], FP32, kind="ExternalInput").ap()
    V = nc.dram_tensor("V", [hpc, S, D], FP32, kind="ExternalInput").ap()
    M = nc.dram_tensor("MASK", [hpc, S], FP32, kind="ExternalInput").ap()
    O = nc.dram_tensor("O", [hpc, S, D], FP32, kind="ExternalOutput").ap()
    dbg = None
    if debug_outputs:
        dbg = {
            "QT": nc.dram_tensor("DBG_QT", [hpc, 128, S // 2], BF16, kind="ExternalOutput").ap(),
            "QTs": nc.dram_tensor("DBG_QTs", [hpc, 128, S // 2], BF16, kind="ExternalOutput").ap(),
            "KT": nc.dram_tensor("DBG_KT", [hpc, 128, S // 2], BF16, kind="ExternalOutput").ap(),
            "pt0": nc.dram_tensor("DBG_pt0", [hpc, 128, S], BF16, kind="ExternalOutput").ap(),
            "pt15": nc.dram_tensor("DBG_pt15", [hpc, 128, S], BF16, kind="ExternalOutput").ap(),
            "rsb": nc.dram_tensor("DBG_rsb", [hpc, 128, 512], FP32, kind="ExternalOutput").ap(),
            "vp": nc.dram_tensor("DBG_vp", [hpc, 128, NCH * D], BF16, kind="ExternalOutput").ap(),
            "mk": nc.dram_tensor("DBG_mk", [hpc, 128, NCH], FP32, kind="ExternalOutput").ap(),
            "vf32": nc.dram_tensor("DBG_vf32", [hpc, 128, NCH * D], FP32, kind="ExternalOutput").ap(),
            "rrec": nc.dram_tensor("DBG_rrec", [hpc, 128, NCH], FP32, kind="ExternalOutput").ap(),
            "osb": nc.dram_tensor("DBG_osb", [hpc, 2, 64, 512], FP32, kind="ExternalOutput").ap(),
        }

    with tile.TileContext(nc) as tc:
        with (
            tc.tile_pool(name="const", bufs=1) as constp,
            tc.tile_pool(name="dram", bufs=2, space="DRAM") as dramp,
            tc.tile_pool(name="heads", bufs=2) as headp,
            tc.tile_pool(name="ptp", bufs=2) as ptp,
            tc.tile_pool(name="pss", bufs=2, space="PSUM") as pss,
            tc.tile_pool(name="psav", bufs=2, space="PSUM") as psav,
            tc.tile_pool(name="pssum", bufs=1, space="PSUM") as pssum,
            tc.tile_pool(name="pstr", bufs=1, space="PSUM") as pstr,
            tc.tile_pool(name="outp", bufs=2) as outp,
            tc.tile_pool(name="smallp", bufs=2) as smallp,
        ):
            ones_bf = constp.tile([128, 1], BF16, tag="ones_bf")
            nc.vector.memset(ones_bf[:], 1.0)
            ident = constp.tile([128, D], FP32, tag="ident")
            nc.gpsimd.memset(ident[:], 0.0)
            from concourse.masks import make_identity
            make_identity(nc, ident[0:64, :], nomemset=True)
            make_identity(nc, ident[64:128, :], nomemset=True)

            pools = (ones_bf, ident, dramp, headp, ptp, pss, psav, pssum, pstr, outp, smallp)
            for h in range(hpc):
                _build_head(tc, pools, O, Q, K, V, M, h, dbg=dbg)

    nc.compile()
    return nc


_CACHE = {}


def _get_nc(hpc=HPC):
    if hpc not in _CACHE:
        _CACHE[hpc] = build_attention(hpc=hpc)
    return _CACHE[hpc]


def _shard(Q, K, V, mask_out):
    B, H, S_, Dk = Q.shape
    BH = B * H
    hpc = BH // N_CORES
    q = np.ascontiguousarray(np.asarray(Q, dtype=np.float32).reshape(BH, S_, Dk))
    k = np.ascontiguousarray(np.asarray(K, dtype=np.float32).reshape(BH, S_, Dk))
    v = np.ascontiguousarray(np.asarray(V, dtype=np.float32).reshape(BH, S_, Dk))
    m = np.ascontiguousarray(np.asarray(mask_out, dtype=np.float32).reshape(BH, S_))
    in_maps = []
    for i in range(N_CORES):
        sl = slice(i * hpc, (i + 1) * hpc)
        in_maps.append({"Q": q[sl], "K": k[sl], "V": v[sl], "MASK": m[sl]})
    return in_maps, hpc


def kernel(Q, K, V, mask_out):
    B, H, S_, Dk = np.asarray(Q).shape
    in_maps, hpc = _shard(Q, K, V, mask_out)
    nc = _get_nc(hpc)
    res = run_bass_kernel_spmd(nc, in_maps, list(range(N_CORES)))
    O = np.concatenate([res.results[i]["O"] for i in range(N_CORES)], axis=0)
    return np.ascontiguousarray(O.reshape(B, H, S_, Dk).astype(np.float32))


def bench(Q, K, V, mask_out, trace=True):
    """Run once with tracing; returns (output, exec_time_ns, profile)."""
    in_maps, hpc = _shard(Q, K, V, mask_out)
    nc = _get_nc(hpc)
    res = run_bass_kernel_spmd(nc, in_maps, list(range(N_CORES)), trace=trace)
    B, H = 4, 12
    O = np.concatenate([res.results[i]["O"] for i in range(N_CORES)], axis=0)
    out = np.ascontiguousarray(O.reshape(B, H, S, D).astype(np.float32))
    return out, res.exec_time_ns, res


# revision 16
# speedup vs baseline: 1.3910x; 1.2983x over previous
"""Trainium2 Bass kernel: batched multi-head attention with post-softmax
multiplicative key-mask (B=4, H=12, S=2048, Dk=Dv=64, fp32 I/O).

out = softmax(Q K^T / 8) * mask(k) @ V  ==  softmax(Q K^T / 8) @ (diag(mask) V)

Sharding: B*H = 48 heads, 6 per NeuronCore across 8 cores (no cross-core
communication).

Per-head on-chip algorithm (S^T layout so no giant transposes are needed):
  - Q, K are cast to bf16 and transposed to [d, q]/[d, k] "parity" layout
    via the DMA xbar transpose ([1024, 128] DRAM view -> [128, 1024] SBUF
    where rows 0:64 = even rows' transpose, 64:128 = odd rows').
  - S^T[k-chunk, q] = (K^T chunk).T @ Q^T tiles on TensorE (bf16, fp32 acc).
  - P^T = exp(S^T / 8) on ScalarE (exact fp32 spline), output bf16.
  - row sums (softmax denominators) = ones.T @ P^T accumulated over chunks
    (column-packed 4-wide on the PE array).
  - out^T[v, q] = sum_c V'[c].T-style accumulation: matmul(lhsT=V'chunk,
    rhs=P^T chunk) accumulated over 16 chunks (column-packed 2-wide),
    where V' = mask(k) * V in bf16.
  - normalize by 1/sums (reciprocal_approx_accurate + gpsimd
    partition_broadcast + one DVE multiply), PE-transpose back to [q, v],
    DMA out.
"""

import sys

for _p in ("/opt/trn_rl_repo", "/root/.axon_site/_ro/trn_rl_repo"):
    if _p not in sys.path:
        sys.path.insert(0, _p)

import numpy as np

import concourse.bass as bass
import concourse.mybir as mybir
import concourse.tile as tile
from concourse import bacc
from concourse.bass_utils import run_bass_kernel_spmd

FP32 = mybir.dt.float32
BF16 = mybir.dt.bfloat16
AF = mybir.ActivationFunctionType

S = 2048
D = 64
NCH = 16          # k chunks of 128
N_CORES = 8
HPC = 6           # heads per core (48 total)


def _build_head(tc, pools, O, Q, K, V, M, h, dbg=None):
    nc = tc.nc
    (ones_bf, ident, dramp, headp, ptp, pss, psav, pssum, pstr, outp, smallp) = pools

    # ---------------- phase 0: load + prep ----------------
    # bf16 staging in DRAM, natural [S, D] layout (gpsimd cast-DMA)
    qbf_d = dramp.tile([S, D], BF16, tag="qbf_d")
    kbf_d = dramp.tile([S, D], BF16, tag="kbf_d")
    nc.gpsimd.dma_start(out=qbf_d[:], in_=Q[h])
    nc.gpsimd.dma_start(out=kbf_d[:], in_=K[h])

    # xbar transpose -> parity layout:
    #   QT[c, r] (c<64):  Q^T[d=c, q=2r]   (even q)
    #   QT[c, r] (c>=64): Q^T[d=c-64, q=2r+1] (odd q)
    QT = headp.tile([128, S // 2], BF16, tag="QT")
    KT = headp.tile([128, S // 2], BF16, tag="KT")
    nc.sync.dma_start_transpose(QT[:], qbf_d[:].rearrange("(r two) d -> r (two d)", two=2))
    nc.sync.dma_start_transpose(KT[:], kbf_d[:].rearrange("(r two) d -> r (two d)", two=2))

    # swapped-half copy of QT (odd-q rows on top), for the cross-parity MMs
    QTs = headp.tile([128, S // 2], BF16, tag="QTs")
    nc.sync.dma_start(out=QTs[0:64, :], in_=QT[64:128, :])
    nc.sync.dma_start(out=QTs[64:128, :], in_=QT[0:64, :])

    if dbg is not None:
        nc.sync.dma_start(out=dbg["QT"][h], in_=QT[:])
        nc.sync.dma_start(out=dbg["QTs"][h], in_=QTs[:])
        nc.sync.dma_start(out=dbg["KT"][h], in_=KT[:])

    # V and mask, k-reordered to parity chunks: chunk c<8 holds k=256c+2i,
    # chunk c>=8 holds k=256(c-8)+2i+1  (i = partition row)
    vf32 = headp.tile([128, NCH * D], FP32, tag="vf32")
    mk = headp.tile([128, NCH], FP32, tag="mk")
    for par in range(2):
        nc.sync.dma_start(
            out=vf32[:, par * 8 * D:(par + 1) * 8 * D].rearrange(
                "p (c8 d) -> p c8 d", d=D),
            in_=V[h].rearrange("(c8 i two) d -> two i c8 d", two=2, i=128)[par],
        )
        nc.gpsimd.dma_start(
            out=mk[:, par * 8:(par + 1) * 8],
            in_=M[h].rearrange("(c8 i two) -> two i c8", two=2, i=128)[par],
        )
    # masked V in bf16 with a ones column appended per chunk (the ones
    # column makes the AV matmul also produce the softmax denominators in
    # psum row 64). mask fold exact: mask is 0.0/1.0.
    DP1 = D + 1
    vp = headp.tile([128, NCH * DP1], BF16, tag="vp")
    for c in range(NCH):
        nc.vector.tensor_scalar_mul(
            vp[:, c * DP1:c * DP1 + D], vf32[:, c * D:(c + 1) * D], mk[:, c:c + 1]
        )
        nc.vector.memset(vp[:, c * DP1 + D:(c + 1) * DP1], 1.0)

    if dbg is not None:
        nc.sync.dma_start(out=dbg["vp"][h], in_=vp[:])
        nc.sync.dma_start(out=dbg["mk"][h], in_=mk[:])
        nc.sync.dma_start(out=dbg["vf32"][h], in_=vf32[:])

    # ---------------- phase 1: S^T = K Q^T, P^T = exp(S^T/8) ----------------
    # P^T stored bf16: cols [c*2048 + 0:1024] even-q (q=2r), [+1024:2048] odd-q
    pt = ptp.tile([128, NCH * S], BF16, tag="pt")
    qk_order = []
    for c8 in range(8):
        for half in range(2):
            qk_order.append((c8, half))
            qk_order.append((c8 + 8, half))
    for c, half in qk_order:
        base = 0 if c < 8 else 64
        lhsT = KT[base:base + 64, (c % 8) * 128:(c % 8 + 1) * 128]
        if True:
            if (half == 0) == (c < 8):
                rhs_t = QT  # natural parity
            else:
                rhs_t = QTs
            st = pss.tile([128, 1024], FP32, tag="st")
            for t in range(2):
                nc.tensor.matmul(
                    st[:, t * 512:(t + 1) * 512],
                    lhsT,
                    rhs_t[base:base + 64, t * 512:(t + 1) * 512],
                    start=True,
                    stop=True,
                    tile_position=(base, 0),
                )
            nc.scalar.activation(
                out=pt[:, c * S + half * 1024: c * S + half * 1024 + 1024],
                in_=st[:],
                func=AF.Exp,
                scale=0.125,
            )

    if dbg is not None:
        nc.sync.dma_start(out=dbg["pt0"][h], in_=pt[:, 0:S])
        nc.sync.dma_start(out=dbg["pt15"][h], in_=pt[:, 15 * S:16 * S])

    # ---------------- phase 2: fused AV+sums, normalize, transpose, store --
    # out rows 0:63 = out^T[v, q], row 64 = softmax denominator (ones col)
    avs = []
    for qt in range(4):
        av = psav.tile([65, 512], FP32, tag="av")
        avs.append(av)
        for c in range(NCH):
            nc.tensor.matmul(
                av[:],
                vp[:, c * DP1:(c + 1) * DP1],
                pt[:, c * S + qt * 512: c * S + (qt + 1) * 512],
                start=(c == 0),
                stop=(c == NCH - 1),
            )

    # softmax denominators: row 64 of each av psum -> SBUF row 64 (lane-
    # aligned), bounce through DRAM into a [128, 16] per-partition layout
    # (col = qt*4 + j, partition = within-j-block index i, matching the
    # transposed output tiles), then one standard reciprocal op.
    rsb = smallp.tile([128, 4 * 512], FP32, tag="rsb")
    for qt in range(4):
        nc.vector.tensor_copy(
            rsb[64:65, qt * 512:(qt + 1) * 512], avs[qt][64:65, :]
        )
    sums_d = dramp.tile([4, 512], FP32, tag="sums_d")
    nc.sync.dma_start(out=sums_d[:].rearrange("a b -> (a b)"), in_=rsb[64:65, :])
    rs128 = smallp.tile([128, NCH], FP32, tag="rs128")
    nc.gpsimd.dma_start(
        out=rs128[:].rearrange("i (qt j) -> i qt j", qt=4),
        in_=sums_d[:].rearrange("qt (j i) -> i qt j", j=4),
    )
    rrec = smallp.tile([128, NCH], FP32, tag="rrec")
    nc.vector.reciprocal(rrec[:], rs128[:])

    if dbg is not None:
        nc.sync.dma_start(out=dbg["rsb"][h], in_=rsb[:])
        nc.sync.dma_start(out=dbg["rrec"][h], in_=rrec[:])

    # copy out^T psum to SBUF (releases the av slot early), PE-transpose
    # back to [q, v], then normalize all 16 transposed tiles with ONE
    # broadcast tensor_tensor multiply by 1/sums at the end.
    ofin = outp.tile([128, NCH * D], FP32, tag="ofin")
    for qt in range(4):
        osb = outp.tile([64, 512], FP32, tag="osb")
        nc.vector.tensor_copy(osb[:], avs[qt][0:64, :])
        if dbg is not None and qt % 2 == 0:
            nc.sync.dma_start(out=dbg["osb"][h, qt // 2], in_=osb[:])
        for j in range(4):
            tr = pstr.tile([128, 64], FP32, tag="tr")
            nc.tensor.transpose(
                tr[:],
                osb[:, j * 128:(j + 1) * 128],
                ident[0:64, :],
            )
            col = qt * 4 + j
            nc.vector.tensor_copy(ofin[:, col * D:(col + 1) * D], tr[:])
    ofin2 = outp.tile([128, NCH * D], FP32, tag="ofin2")
    nc.vector.tensor_tensor(
        out=ofin2[:].rearrange("p (c v) -> p c v", v=D),
        in0=ofin[:].rearrange("p (c v) -> p c v", v=D),
        in1=rrec[:, :, None].broadcast_to([128, NCH, D]),
        op=mybir.AluOpType.mult,
    )

    # store: ofin col (qt*4+j)*64+v, partition i  ->  O[h, q, v],
    # q = 2*((qt%2)*512 + j*128 + i) + (qt//2)
    for half in range(2):
        nc.sync.dma_start(
            out=O[h].rearrange("(qt j i two) v -> two i qt j v", qt=2, j=4, two=2)[half],
            in_=ofin2[:].rearrange("i (half qt j v) -> half i qt j v", half=2, qt=2, j=4)[half],
        )


def build_attention(hpc=HPC, n_cores=N_CORES, trn_type="TRN2", debug_outputs=False):
    nc = bacc.Bacc(
        trn_type,
        target_bir_lowering=False,
        debug=False,
        num_devices=n_cores,
    )
    Q = nc.dram_tensor("Q", [hpc, S, D], FP32, kind="ExternalInput").ap()
    K = nc.dram_tensor("K", [hpc, S, D], FP32, kind="ExternalInput").ap()
    V = nc.dram_tensor("V", [hpc, S, D], FP32, kind="ExternalInput").ap()
    M = nc.dram_tensor("MASK", [hpc, S], FP32, kind="ExternalInput").ap()
    O = nc.dram_tensor("O", [hpc, S, D], FP32, kind="ExternalOutput").ap()
    dbg = None
    if debug_outputs:
        dbg = {
            "QT": nc.dram_tensor("DBG_QT", [hpc, 128, S // 2], BF16, kind="ExternalOutput").ap(),
            "QTs": nc.dram_tensor("DBG_QTs", [hpc, 128, S // 2], BF16, kind="ExternalOutput").ap(),
            "KT": nc.dram_tensor("DBG_KT", [hpc, 128, S // 2], BF16, kind="ExternalOutput").ap(),
            "pt0": nc.dram_tensor("DBG_pt0", [hpc, 128, S], BF16, kind="ExternalOutput").ap(),
            "pt15": nc.dram_tensor("DBG_pt15", [hpc, 128, S], BF16, kind="ExternalOutput").ap(),
            "rsb": nc.dram_tensor("DBG_rsb", [hpc, 128, 512], FP32, kind="ExternalOutput").ap(),
            "vp": nc.dram_tensor("DBG_vp", [hpc, 128, NCH * D], BF16, kind="ExternalOutput").ap(),
            "mk": nc.dram_tensor("DBG_mk", [hpc, 128, NCH], FP32, kind="ExternalOutput").ap(),
            "vf32": nc.dram_tensor("DBG_vf32", [hpc, 128, NCH * D], FP32, kind="ExternalOutput").ap(),
            "rrec": nc.dram_tensor("DBG_rrec", [hpc, 128, NCH], FP32, kind="ExternalOutput").ap(),
            "osb": nc.dram_tensor("DBG_osb", [hpc, 2, 64, 512], FP32, kind="ExternalOutput").ap(),
        }

    with tile.TileContext(nc) as tc:
        with (
            tc.tile_pool(name="const", bufs=1) as constp,
            tc.tile_pool(name="dram", bufs=2, space="DRAM") as dramp,
            tc.tile_pool(name="heads", bufs=2) as headp,
            tc.tile_pool(name="ptp", bufs=2) as ptp,
            tc.tile_pool(name="pss", bufs=2, space="PSUM") as pss,
            tc.tile_pool(name="psav", bufs=3, space="PSUM") as psav,
            tc.tile_pool(name="pstr", bufs=1, space="PSUM") as pstr,
            tc.tile_pool(name="outp", bufs=2) as outp,
            tc.tile_pool(name="smallp", bufs=2) as smallp,
        ):
            ones_bf = constp.tile([128, 1], BF16, tag="ones_bf")
            nc.vector.memset(ones_bf[:], 1.0)
            ident = constp.tile([128, D], FP32, tag="ident")
            nc.gpsimd.memset(ident[:], 0.0)
            from concourse.masks import make_identity
            make_identity(nc, ident[0:64, :], nomemset=True)
            make_identity(nc, ident[64:128, :], nomemset=True)

            pools = (ones_bf, ident, dramp, headp, ptp, pss, psav, None, pstr, outp, smallp)
            for h in range(hpc):
                _build_head(tc, pools, O, Q, K, V, M, h, dbg=dbg)

    nc.compile()
    return nc


_CACHE = {}


def _get_nc(hpc=HPC):
    if hpc not in _CACHE:
        _CACHE[hpc] = build_attention(hpc=hpc)
    return _CACHE[hpc]


def _shard(Q, K, V, mask_out):
    B, H, S_, Dk = Q.shape
    BH = B * H
    hpc = BH // N_CORES
    q = np.ascontiguousarray(np.asarray(Q, dtype=np.float32).reshape(BH, S_, Dk))
    k = np.ascontiguousarray(np.asarray(K, dtype=np.float32).reshape(BH, S_, Dk))
    v = np.ascontiguousarray(np.asarray(V, dtype=np.float32).reshape(BH, S_, Dk))
    m = np.ascontiguousarray(np.asarray(mask_out, dtype=np.float32).reshape(BH, S_))
    in_maps = []
    for i in range(N_CORES):
        sl = slice(i * hpc, (i + 1) * hpc)
        in_maps.append({"Q": q[sl], "K": k[sl], "V": v[sl], "MASK": m[sl]})
    return in_maps, hpc


def kernel(Q, K, V, mask_out):
    B, H, S_, Dk = np.asarray(Q).shape
    in_maps, hpc = _shard(Q, K, V, mask_out)
    nc = _get_nc(hpc)
    res = run_bass_kernel_spmd(nc, in_maps, list(range(N_CORES)))
    O = np.concatenate([res.results[i]["O"] for i in range(N_CORES)], axis=0)
    return np.ascontiguousarray(O.reshape(B, H, S_, Dk).astype(np.float32))


def bench(Q, K, V, mask_out, trace=True):
    """Run once with tracing; returns (output, exec_time_ns, profile)."""
    in_maps, hpc = _shard(Q, K, V, mask_out)
    nc = _get_nc(hpc)
    res = run_bass_kernel_spmd(nc, in_maps, list(range(N_CORES)), trace=trace)
    B, H = 4, 12
    O = np.concatenate([res.results[i]["O"] for i in range(N_CORES)], axis=0)
    out = np.ascontiguousarray(O.reshape(B, H, S, D).astype(np.float32))
    return out, res.exec_time_ns, res
